# revision 3
# baseline (speedup 1.0000x reference)
"""Trainium2 Bass kernel for nn_CliffordJEPAModel.

Model = two GRU encoders (ctx / tgt) + tiny closed-form head.

Key observations:
  * The energy-descent loop is linear in h (grad is constant), so
    pred_latent = -0.5 * ctx_latent @ Wsn^T  in closed form.
  * The heavy work is two 256-step GRUs (B=64, D=768). Each recurrent
    step is weight-ingest bound on the PE array, independent of local
    batch size, so we shard: 8 cores = 2 encoders x 4 batch-quarters
    (B_local=16), no cross-core communication.
  * Everything is laid out "gates on partitions" (orientation: out^T =
    W @ x^T) so the per-step gate math runs on full 128-partition tiles.
  * Embedding gather uses dma_gather(transpose=True) which directly
    produces the transposed X^T layout the matmuls need.

Per-core program (identical on all 8 cores; only input DATA differs):
  phase 1+2: gather X^T chunks and compute gi^T = Wih' @ X^T + bias
             (gate rows permuted into [r,z,n]-interleaved m-tiles),
             stored to a DRAM scratch.
  phase 3:   256 sequential GRU steps:
             gh^T = Whh' @ h^T   (18 m-tiles of 128 gate rows, N=16)
             gates on DVE/ACT, h ping-pong in fp32 (+bf16 copy for PE).
  output:    final h^T  [128, 6*16] fp32.

Host does the final tiny head math in numpy (fc -> spectral norm ->
closed-form descent), all O(64*768*8) flops.
"""

import os
import sys

for _p in ("/opt/trn_rl_repo/concourse", "/opt/trn_rl_repo"):
    if _p not in sys.path:
        sys.path.insert(0, _p)

import numpy as np
import ml_dtypes

import concourse.bacc as bacc
import concourse.mybir as mybir
import concourse.tile as tile
from concourse.bass_utils import run_bass_kernel_spmd

BF16 = ml_dtypes.bfloat16

V, D, NB = 32000, 768, 8
B, S = 64, 256
DT_STEP, STEPS_DESC, PI = 0.1, 5, 3

N_CORES = 8
BQ = B // 4          # batch rows per core (16)
KT = D // 128        # 6 k-tiles
MT = 3 * KT          # 18 m-tiles of gate rows
NT = BQ * S          # tokens per core (4096)
CHT = 512            # tokens per gather/input-matmul chunk
NCH = NT // CHT      # 8 chunks
BLK = 16             # recurrence steps per gi prefetch block
NBLK = S // BLK

F32 = mybir.dt.float32
BF16_T = mybir.dt.bfloat16
I16 = mybir.dt.int16
AF = mybir.ActivationFunctionType

# gate-row permutation: m-tile j = (chunk c=j//3, gate g=j%3) covers rows
# g*768 + c*128 .. +128  ->  interleaved [r_c, z_c, n_c] blocks.
_PERM = np.concatenate(
    [np.arange(g * D + c * 128, g * D + (c + 1) * 128) for c in range(KT) for g in range(3)]
)


def _build_program(steps=S):
    nc = bacc.Bacc("TRN2", target_bir_lowering=False, debug=False, num_devices=N_CORES)

    t_idx = nc.dram_tensor("idx", [128, NT // 16], I16, kind="ExternalInput")
    t_emb = nc.dram_tensor("emb", [V, D], BF16_T, kind="ExternalInput")
    t_wih = nc.dram_tensor("wihT", [128, KT * 3 * D], BF16_T, kind="ExternalInput")
    t_whh = nc.dram_tensor("whhT", [128, KT * 3 * D], BF16_T, kind="ExternalInput")
    t_bi = nc.dram_tensor("bias_i", [128, MT], F32, kind="ExternalInput")
    t_bn = nc.dram_tensor("bhhn", [128, KT * BQ], F32, kind="ExternalInput")
    t_out = nc.dram_tensor("h_out", [128, KT * BQ], F32, kind="ExternalOutput")

    W3D = 3 * D  # 2304

    with tile.TileContext(nc) as tc:
        with (
            tc.tile_pool(name="const", bufs=1) as const_pool,
            tc.tile_pool(name="dram", bufs=1, space="DRAM") as dram_pool,
        ):
            idx_t = const_pool.tile([128, NT // 16], I16)
            wih_t = const_pool.tile([128, KT * W3D], BF16_T)
            whh_t = const_pool.tile([128, KT * W3D], BF16_T)
            bi_t = const_pool.tile([128, MT], F32)
            bn_t = const_pool.tile([128, KT * BQ], F32)
            nc.sync.dma_start(idx_t[:], t_idx.ap())
            nc.sync.dma_start(wih_t[:], t_wih.ap())
            nc.sync.dma_start(whh_t[:], t_whh.ap())
            nc.sync.dma_start(bi_t[:], t_bi.ap())
            nc.sync.dma_start(bn_t[:], t_bn.ap())

            giD = dram_pool.tile([MT, 128, NT], BF16_T)

            # ---- phase 1+2: gather + input matmul -> giD ----
            with (
                tc.tile_pool(name="xt", bufs=3) as xt_pool,
                tc.tile_pool(name="psum_in", bufs=4, space="PSUM") as psum_in,
                tc.tile_pool(name="gis", bufs=4) as gis_pool,
            ):
                for nch in range(NCH):
                    xt = xt_pool.tile([128, KT, CHT], BF16_T)
                    nc.gpsimd.dma_gather(
                        xt[:, :, :],
                        t_emb.ap(),
                        idx_t[:, nch * (CHT // 16):(nch + 1) * (CHT // 16)],
                        num_idxs=CHT,
                        num_idxs_reg=CHT,
                        elem_size=D,
                        transpose=True,
                    )
                    for m in range(MT):
                        ps = psum_in.tile([128, CHT], F32)
                        for k in range(KT):
                            nc.tensor.matmul(
                                ps[:],
                                wih_t[:, k * W3D + m * 128:k * W3D + (m + 1) * 128],
                                xt[:, k, :],
                                start=(k == 0),
                                stop=(k == KT - 1),
                            )
                        gs = gis_pool.tile([128, CHT], BF16_T)
                        nc.scalar.activation(gs[:], ps[:], AF.Identity, bias=bi_t[:, m:m + 1], scale=1.0)
                        nc.sync.dma_start(giD[m, :, nch * CHT:(nch + 1) * CHT], gs[:])

            # ---- phase 3: recurrence ----
            with (
                tc.tile_pool(name="gh", bufs=8, space="PSUM") as gh_pool,
                tc.tile_pool(name="giblk", bufs=2) as giblk_pool,
                tc.tile_pool(name="hstate", bufs=1) as h_pool,
                tc.tile_pool(name="tmp", bufs=4) as tmp,
            ):
                h_f = [h_pool.tile([128, KT * BQ], F32, name=f"hf{i}", tag=f"hf{i}") for i in range(2)]
                h_b = [h_pool.tile([128, KT * BQ], BF16_T, name=f"hb{i}", tag=f"hb{i}") for i in range(2)]
                nc.vector.memset(h_f[0][:], 0.0)
                nc.vector.memset(h_b[0][:], 0.0)

                nblk = steps // BLK
                for blk in range(nblk):
                    gi_blk = giblk_pool.tile([128, BLK, MT * BQ], BF16_T)
                    for m in range(MT):
                        nc.sync.dma_start(
                            gi_blk[:, :, m * BQ:(m + 1) * BQ],
                            giD[m, :, blk * BLK * BQ:(blk + 1) * BLK * BQ].rearrange(
                                "p (t b) -> p t b", b=BQ
                            ),
                        )
                    for tl in range(BLK):
                        t = blk * BLK + tl
                        cur, nxt = t % 2, (t + 1) % 2
                        gh = [gh_pool.tile([128, 3 * BQ], F32, name="gh", tag="gh") for _ in range(KT)]
                        for c in range(KT):
                            for k in range(KT):
                                for g in range(3):
                                    nc.tensor.matmul(
                                        gh[c][:, g * BQ:(g + 1) * BQ],
                                        whh_t[:, k * W3D + (3 * c + g) * 128:k * W3D + (3 * c + g + 1) * 128],
                                        h_b[cur][:, k * BQ:(k + 1) * BQ],
                                        start=(k == 0 and g == 0),
                                        stop=(k == KT - 1 and g == 2),
                                    )
                        for c in range(KT):
                            c0 = 3 * c * BQ  # gi col offset of this chunk
                            arz = tmp.tile([128, 2 * BQ], F32, tag="arz")
                            nc.vector.tensor_add(arz[:], gh[c][:, 0:2 * BQ], gi_blk[:, tl, c0:c0 + 2 * BQ])
                            rz = tmp.tile([128, 2 * BQ], F32, tag="rz")
                            nc.scalar.activation(rz[:], arz[:], AF.Sigmoid)
                            hn = tmp.tile([128, BQ], F32, tag="hn")
                            nc.vector.tensor_add(hn[:], gh[c][:, 2 * BQ:3 * BQ], bn_t[:, c * BQ:(c + 1) * BQ])
                            u = tmp.tile([128, BQ], F32, tag="u")
                            nc.vector.tensor_mul(u[:], rz[:, 0:BQ], hn[:])
                            v = tmp.tile([128, BQ], F32, tag="v")
                            nc.vector.tensor_add(v[:], u[:], gi_blk[:, tl, c0 + 2 * BQ:c0 + 3 * BQ])
                            n_t = tmp.tile([128, BQ], F32, tag="n")
                            nc.scalar.activation(n_t[:], v[:], AF.Tanh)
                            d_t = tmp.tile([128, BQ], F32, tag="d")
                            nc.vector.tensor_sub(d_t[:], h_f[cur][:, c * BQ:(c + 1) * BQ], n_t[:])
                            e_t = tmp.tile([128, BQ], F32, tag="e")
                            nc.vector.tensor_mul(e_t[:], rz[:, BQ:2 * BQ], d_t[:])
                            nc.vector.tensor_add(h_f[nxt][:, c * BQ:(c + 1) * BQ], n_t[:], e_t[:])
                            nc.vector.tensor_copy(h_b[nxt][:, c * BQ:(c + 1) * BQ], h_f[nxt][:, c * BQ:(c + 1) * BQ])

                nc.sync.dma_start(t_out.ap(), h_f[steps % 2][:])

    nc.compile()
    return nc


def _pack_encoder(emb, Wih, Whh, bih, bhh):
    """Host-side prep of one encoder's parameters into device layouts."""
    emb_bf = np.ascontiguousarray(emb.astype(BF16))
    Wp = Wih[_PERM]  # [2304, 768]
    wihT = np.ascontiguousarray(
        Wp.reshape(3 * D, KT, 128).transpose(2, 1, 0).reshape(128, KT * 3 * D).astype(BF16)
    )
    Wp = Whh[_PERM]
    whhT = np.ascontiguousarray(
        Wp.reshape(3 * D, KT, 128).transpose(2, 1, 0).reshape(128, KT * 3 * D).astype(BF16)
    )
    bias_vec = (bih + np.concatenate([bhh[:D], bhh[D:2 * D], np.zeros(D, np.float32)]))[_PERM]
    bias_i = np.ascontiguousarray(bias_vec.reshape(MT, 128).T.astype(np.float32))
    bhh_n = bhh[2 * D:]
    bhhn = np.ascontiguousarray(
        np.repeat(bhh_n.reshape(KT, 128).T[:, :, None], BQ, axis=2).reshape(128, KT * BQ).astype(np.float32)
    )
    return emb_bf, wihT, whhT, bias_i, bhhn


_CACHE = {}


def run_device(inputs, steps=S, trace=False):
    """Run the 8-core device program; returns (h_ctx [64,768], h_tgt [64,768], perf)."""
    key = steps
    if key not in _CACHE:
        _CACHE[key] = _build_program(steps)
    nc = _CACHE[key]

    ctx_tok = np.asarray(inputs["ctx"]).astype(np.int16)      # [64, 256]
    tgt_tok = np.asarray(inputs["tgt_seq"]).astype(np.int16)  # [64, 256]

    enc_ctx = _pack_encoder(
        np.asarray(inputs["emb"], np.float32), np.asarray(inputs["Wih"], np.float32),
        np.asarray(inputs["Whh"], np.float32), np.asarray(inputs["bih"], np.float32),
        np.asarray(inputs["bhh"], np.float32),
    )
    enc_tgt = _pack_encoder(
        np.asarray(inputs["t_emb"], np.float32), np.asarray(inputs["t_Wih"], np.float32),
        np.asarray(inputs["t_Whh"], np.float32), np.asarray(inputs["t_bih"], np.float32),
        np.asarray(inputs["t_bhh"], np.float32),
    )

    in_maps = []
    for core in range(N_CORES):
        e, q = core // 4, core % 4
        emb_bf, wihT, whhT, bias_i, bhhn = enc_ctx if e == 0 else enc_tgt
        toks = (ctx_tok if e == 0 else tgt_tok)[q * BQ:(q + 1) * BQ, :]  # [16, 256]
        # gather position i = t*16+b reads idx[i%16, i//16] = toks[b, t]; the
        # [16, NT/16] block must be replicated into each gpsimd core's stripe.
        idx = np.tile(toks, (8, 1))
        in_maps.append({
            "idx": idx,
            "emb": emb_bf,
            "wihT": wihT,
            "whhT": whhT,
            "bias_i": bias_i,
            "bhhn": bhhn,
        })

    res = run_bass_kernel_spmd(nc, in_maps, core_ids=list(range(N_CORES)), trace=trace)

    def unpack_h(outs):
        # out [128, KT*BQ]: out[p, k*BQ + b] = h[b, k*128 + p]
        h = np.zeros((4 * BQ, D), np.float32)
        for q in range(4):
            o = outs[q]["h_out"].reshape(128, KT, BQ)
            h[q * BQ:(q + 1) * BQ, :] = o.transpose(2, 1, 0).reshape(BQ, D)
        return h

    h_ctx = unpack_h(res.results[0:4])
    h_tgt = unpack_h(res.results[4:8])
    return h_ctx, h_tgt, res


def _head(h_ctx, h_tgt, inputs):
    """Final tiny math on host, float64 for exactness."""
    Wfc = np.asarray(inputs["Wfc"], np.float64)
    bfc = np.asarray(inputs["bfc"], np.float64)
    tWfc = np.asarray(inputs["t_Wfc"], np.float64)
    tbfc = np.asarray(inputs["t_bfc"], np.float64)
    We = np.asarray(inputs["We"], np.float64)
    u0 = np.asarray(inputs["u_sn"], np.float64)

    ctx_latent = h_ctx.astype(np.float64) @ Wfc.T + bfc          # [64, 8]
    target_latent = h_tgt.astype(np.float64) @ tWfc.T + tbfc     # [64, 8]

    u = u0 / (np.linalg.norm(u0) + 1e-12)
    for _ in range(PI):
        v = We.T @ u
        v = v / (np.linalg.norm(v) + 1e-12)
        u = We @ v
        u = u / (np.linalg.norm(u) + 1e-12)
    sigma = u @ (We @ v)
    Wsn = We / sigma

    pred_latent = -(STEPS_DESC * DT_STEP) * (ctx_latent @ Wsn.T)  # [64, 8]
    return (
        pred_latent.astype(np.float32)[:, None, :],
        target_latent.astype(np.float32)[:, None, :],
    )


def kernel(**inputs):
    h_ctx, h_tgt, _ = run_device(inputs, steps=S, trace=False)
    return _head(h_ctx, h_tgt, inputs)
